# revision 20
# baseline (speedup 1.0000x reference)
"""FSQ codebook kernel for Trainium2 (8 NeuronCores, data-parallel over tokens).

Computes, for x:(8,8192,1280) f32, W:(8,1280) f32, b:(8,) f32:
    h  = x.reshape(-1,1280) @ W.T + b            # (65536, 8)
    mu = sum_k 3^k * (1 + round(tanh(h)*SCALE))  # base-3 code, int32
    -> (8, 8192) int32

The tanh/round/scale pipeline is replaced by an exact fp32 threshold:
    round(tanh(h)*SCALE) = +1  iff  h >= T_POS
                         = -1  iff  h <= -T_POS      (bit-exact, verified)
so digit value (1+r) = [h >= T_POS] + [h > -T_POS] and
    mu = sum_k 3^k*[h_k >= T] + sum_k 3^k*[h_k > -T].

fp16x2 path: x and W are scaled by 2^10 and Dekker-split on the host into
hi/lo fp16 pairs (x*2^10 = hi + lo + O(2^-24); same total bytes as fp32).
The host ALSO pre-transposes the per-core shards to (D, TOK) so the kernel
loads x with d on partitions via plain contiguous DMA (2 KB descriptors) —
no XBAR transpose DMA (245 B descriptors, ~60% of peak) and no PE
transposes. The GEMM accumulates the 2^20-scaled h in fp32 PSUM via a
stacked [Whi|Wlo] stationary (2 matmuls per d-tile cover all 4 Dekker
products).

The bias add + threshold compare is folded into per-k f32 thresholds
computed exactly on the host by monotone bisection over f32
(digit+ = [h >= tpos_k] <=> [fl32(h + 2^20 b_k) >= 2^20 T_POS], exact for
every representable h), removing 2 matmuls + 2 LDWEIGHTS per 512 tokens.
Const loads ride the scalar HWDGE ring so the x stream owns the sync ring
from cycle 0; the last group's loads are split in half to shorten the
serial tail; mu is written back per 512-token slice (scalar ring) so the
final DMA only covers the last slice.

Measured on trn2 (8 cores, core-0 NEFF span): 262 us (baseline XBAR-
transpose version) -> 153 us; DMA floor for the 42 MB/core stream is
~117 us, PE floor ~82 us (+50%-duty throttle periods).
"""

import numpy as np

# exact fp32 threshold: minimal fp32 v with round(tanh(v)*SCALE) == 1
T_POS = float(np.uint32(0x3F0CCB15).view(np.float32))
SPLIT_SCALE = 1024.0  # 2^10 per operand; h is scaled by 2^20

N_CORES = 8
TOK_PER_CORE = 8192
D = 1280
K = 8
D_TILES = D // 128            # 10

# fp16x2-path tiling: 1024-token groups, matmul N=512 halves
GTOK = 1024
N_GROUP = TOK_PER_CORE // GTOK  # 8

_cached = {}


def _build_fp16x2(repeat=1):
    from contextlib import ExitStack

    from concourse import bacc, mybir, tile

    f16 = mybir.dt.float16
    f32 = mybir.dt.float32
    i32 = mybir.dt.int32

    nc = bacc.Bacc("TRN2", target_bir_lowering=False, debug=False)

    # host-pre-transposed: hi rows then lo rows, (2*D, TOK) contiguous
    xt_ap = nc.dram_tensor(
        "xtall", [2 * D, TOK_PER_CORE], f16, kind="ExternalInput"
    ).ap()
    wpair_ap = nc.dram_tensor(
        "wpair", [128, 400], f16, kind="ExternalInput"
    ).ap()
    tpos_ap = nc.dram_tensor("tpos", [K, 1], f32, kind="ExternalInput").ap()
    tneg_ap = nc.dram_tensor("tneg", [K, 1], f32, kind="ExternalInput").ap()
    pw_ap = nc.dram_tensor("powers", [K, 1], f32, kind="ExternalInput").ap()
    out_ap = nc.dram_tensor(
        "out", [1, TOK_PER_CORE], i32, kind="ExternalOutput"
    ).ap()

    with tile.TileContext(nc) as tc, ExitStack() as ctx:
        const_pool = ctx.enter_context(tc.tile_pool(name="const", bufs=1))
        xt_pool = ctx.enter_context(tc.tile_pool(name="xt", bufs=3))
        val_pool = ctx.enter_context(tc.tile_pool(name="val", bufs=3))
        mu_pool = ctx.enter_context(tc.tile_pool(name="mu", bufs=1))
        ps_h = ctx.enter_context(tc.tile_pool(name="ps_h", bufs=6, space="PSUM"))
        ps_m = ctx.enter_context(tc.tile_pool(name="ps_m", bufs=2, space="PSUM"))

        # stacked stationary, 40 cols per d-tile: cols [0:8]=Whi_dt,
        # [32:40]=Wlo_dt (partition windows must start at multiples of 32;
        # the unused middle columns cost nothing — matmul time is N-bound).
        # The full SBUF image (with zero gaps) is prebuilt on the host so it
        # loads as ONE contiguous DMA with no memset dependency.
        WP = 40
        wpair_sb = const_pool.tile([128, D_TILES * WP], f16)
        nc.scalar.dma_start(wpair_sb[:], wpair_ap[:])
        tpos_sb = const_pool.tile([K, 1], f32)
        nc.scalar.dma_start(tpos_sb[:], tpos_ap[:])
        tneg_sb = const_pool.tile([K, 1], f32)
        nc.scalar.dma_start(tneg_sb[:], tneg_ap[:])
        pw_sb = const_pool.tile([K, 1], f32)
        nc.scalar.dma_start(pw_sb[:], pw_ap[:])

        mu_i32 = mu_pool.tile([1, TOK_PER_CORE], i32)

        # d-tiles 0-9 = hi, 10-19 = lo
        xt_v = xt_ap.rearrange("(dt p) T -> p dt T", p=128)

        for _rep in range(repeat):
            for g in range(N_GROUP):
                t0 = g * GTOK
                # one load per group: xt[p, dt2, t] = xT[dt2*128+p, t0+t]
                xt = xt_pool.tile([128, 2 * D_TILES, GTOK], f16, name="xt")
                if g == N_GROUP - 1:
                    # split the final loads so the last group's first-half
                    # compute overlaps the second half's DMA (shorter tail)
                    for hh in range(2):
                        ts_ = slice(t0 + hh * 512, t0 + (hh + 1) * 512)
                        nc.sync.dma_start(
                            xt[:, :, hh * 512 : (hh + 1) * 512],
                            xt_v[:, :, ts_],
                        )
                else:
                    nc.sync.dma_start(xt[:], xt_v[:, :, t0 : t0 + GTOK])
                for half in range(2):
                    hs = slice(half * 512, half * 512 + 512)
                    # h40 rows 0-7 += Whi^T@(xthi+xtlo); rows 32-39 += Wlo^T@(...)
                    # all 4 Dekker products in 2 matmuls per d-tile
                    h40 = ps_h.tile([WP, 512], f32)
                    mm = [
                        (dt, half_lo)
                        for dt in range(D_TILES)
                        for half_lo in (0, 1)
                    ]
                    for i, (dt, half_lo) in enumerate(mm):
                        nc.tensor.matmul(
                            h40[:],
                            lhsT=wpair_sb[:, dt * WP : (dt + 1) * WP],
                            rhs=xt[:, half_lo * D_TILES + dt, hs],
                            start=(i == 0),
                            stop=(i == len(mm) - 1),
                        )

                    # h = rows[0:8] + rows[32:40]; val = [h >= T] + [h > -T]
                    # (tensor_tensor may read only one PSUM operand)
                    hlo_sb = val_pool.tile([K, 512], f32, name="hlo_sb")
                    nc.vector.tensor_copy(hlo_sb[:], h40[32 : 32 + K, :])
                    hsum = val_pool.tile([K, 512], f32, name="hsum")
                    nc.vector.tensor_add(hsum[:], h40[0:K, :], hlo_sb[:])
                    val1 = val_pool.tile([K, 512], f32, name="val1")
                    nc.vector.tensor_scalar(
                        out=val1[:],
                        in0=hsum[:],
                        scalar1=tpos_sb[:, 0:1],
                        scalar2=None,
                        op0=mybir.AluOpType.is_ge,
                    )
                    val = val_pool.tile([K, 512], f32, name="val")
                    nc.vector.scalar_tensor_tensor(
                        out=val[:],
                        in0=hsum[:],
                        scalar=tneg_sb[:, 0:1],
                        in1=val1[:],
                        op0=mybir.AluOpType.is_ge,
                        op1=mybir.AluOpType.add,
                    )
                    # mu = powers^T @ val   (K=8 contraction)
                    mu_ps = ps_m.tile([1, 512], f32)
                    nc.tensor.matmul(
                        mu_ps[:], lhsT=pw_sb[:], rhs=val[:], start=True, stop=True
                    )
                    base = t0 + half * 512
                    nc.vector.tensor_copy(
                        mu_i32[:, base : base + 512], mu_ps[:]
                    )
                    nc.scalar.dma_start(
                        out_ap[:, base : base + 512],
                        mu_i32[:, base : base + 512],
                    )

    nc.compile()
    return nc



# ---- two-pass mode (EXPERIMENTAL, correct but slower: 229 us vs 153 us) ----
# Streams only the fp16-hi half (21 MB), flags tokens whose scaled logits
# land within EPS_S of a decision threshold (mu+ != mu- under -/+eps-shifted
# thresholds), compacts flagged 4-token chunks on-device (scan + ones-matmul
# broadcast + per-partition compare), gathers them via indirect DMA, exactly
# recomputes with the full Dekker product and scatters the fixes (ordered by
# chain_iter_dep). Verified exact (0/65536 mismatches) but loses to the
# single-pass: the 160 PE transposes needed to turn gathered token-major rows
# into d-major matmul operands (+ their PSUM->SBUF copies) push the PE, which
# this part runs 50%-duty-throttled, to ~134 us active - past the 55 us DMA
# stream it was meant to hide. To win, the transpose must leave the PE (e.g.
# gather -> DRAM restage -> one XBAR transpose DMA per batch).
EPS_S = 1600.0        # scaled flag window; measured max |h_A - h_true| = 1418
NCHUNK = 2048         # 4-token chunks per core
NSLOT = 256           # repair budget (2 batches of 128); measured need <= 159
BIGIDX = 1 << 20


def _build_twopass(repeat=1):
    from contextlib import ExitStack

    from concourse import bacc, bass as _bass, masks, mybir, tile

    f16 = mybir.dt.float16
    f32 = mybir.dt.float32
    i32 = mybir.dt.int32

    nc = bacc.Bacc("TRN2", target_bir_lowering=False, debug=False)

    xthi_ap = nc.dram_tensor("xthi", [D, TOK_PER_CORE], f16, kind="ExternalInput").ap()
    xpair_ap = nc.dram_tensor(
        "xpair", [NCHUNK, 4 * 2 * D], f16, kind="ExternalInput"
    ).ap()
    wthi_ap = nc.dram_tensor("wthi", [D, K], f16, kind="ExternalInput").ap()
    wtlo_ap = nc.dram_tensor("wtlo", [D, K], f16, kind="ExternalInput").ap()
    thr4_ap = nc.dram_tensor("thr4", [K, 4], f32, kind="ExternalInput").ap()
    thr2_ap = nc.dram_tensor("thr2", [K, 2], f32, kind="ExternalInput").ap()
    pw2_ap = nc.dram_tensor("pw2", [40, 33], f32, kind="ExternalInput").ap()
    iop1_ap = nc.dram_tensor("iop1", [1, NCHUNK], f32, kind="ExternalInput").ap()
    jcol_ap = nc.dram_tensor("jcol", [128, 2], f32, kind="ExternalInput").ap()
    muscr_ap = nc.dram_tensor("muscr", [2, 512], i32, kind="Internal").ap()
    out_ap = nc.dram_tensor(
        "out", [1, TOK_PER_CORE], i32, kind="ExternalOutput"
    ).ap()
    dbg_ap = nc.dram_tensor("dbg", [128, 2], f32, kind="ExternalOutput").ap()

    with tile.TileContext(nc) as tc, ExitStack() as ctx:
        const_pool = ctx.enter_context(tc.tile_pool(name="const", bufs=1))
        xt_pool = ctx.enter_context(tc.tile_pool(name="xt", bufs=2))
        val_pool = ctx.enter_context(tc.tile_pool(name="val", bufs=2))
        valb_pool = ctx.enter_context(tc.tile_pool(name="valb", bufs=1))
        mu_pool = ctx.enter_context(tc.tile_pool(name="mu", bufs=1))
        cmp_pool = ctx.enter_context(tc.tile_pool(name="cmp", bufs=1))
        xg_pool = ctx.enter_context(tc.tile_pool(name="xg", bufs=2))
        xgt_pool = ctx.enter_context(tc.tile_pool(name="xgt", bufs=1))
        ps_h = ctx.enter_context(tc.tile_pool(name="ps_h", bufs=2, space="PSUM"))
        ps_m = ctx.enter_context(tc.tile_pool(name="ps_m", bufs=1, space="PSUM"))
        ps_c = ctx.enter_context(tc.tile_pool(name="ps_c", bufs=1, space="PSUM"))
        ps_t = ctx.enter_context(tc.tile_pool(name="ps_t", bufs=2, space="PSUM"))

        # pass-A stationary: Whi only, [128, 10*8] f16
        whi_sb = const_pool.tile([128, D_TILES * K], f16)
        nc.scalar.dma_start(
            whi_sb[:].rearrange("p (dt k) -> p dt k", dt=D_TILES),
            wthi_ap.rearrange("(dt p) k -> p dt k", p=128),
        )
        # pass-B stacked stationary [Whi | Wlo] in 40-col windows
        WP = 40
        wpair_sb = const_pool.tile([128, D_TILES * WP], f16)
        nc.vector.memset(wpair_sb[:], 0)
        nc.scalar.dma_start(
            wpair_sb[:].rearrange("p (dt c) -> p dt c", dt=D_TILES)[:, :, 0:K],
            wthi_ap.rearrange("(dt p) k -> p dt k", p=128),
        )
        nc.scalar.dma_start(
            wpair_sb[:].rearrange("p (dt c) -> p dt c", dt=D_TILES)[
                :, :, 32 : 32 + K
            ],
            wtlo_ap.rearrange("(dt p) k -> p dt k", p=128),
        )
        thr4_sb = const_pool.tile([K, 4], f32)
        nc.scalar.dma_start(thr4_sb[:], thr4_ap[:])
        thr2_sb = const_pool.tile([K, 2], f32)
        nc.scalar.dma_start(thr2_sb[:], thr2_ap[:])
        pw2_sb = const_pool.tile([40, 33], f32)
        nc.scalar.dma_start(pw2_sb[:], pw2_ap[:])
        iop1 = const_pool.tile([1, NCHUNK], f32)
        nc.scalar.dma_start(iop1[:], iop1_ap[:])
        jcol = const_pool.tile([128, 2], f32)
        nc.scalar.dma_start(jcol[:], jcol_ap[:])
        ones128 = const_pool.tile([1, 128], f32)
        nc.vector.memset(ones128[:], 1.0)
        ident16 = const_pool.tile([128, 128], f16)
        masks.make_identity(nc, ident16[:])

        cf_all = mu_pool.tile([1, NCHUNK], f32)

        xthi_v = xthi_ap.rearrange("(dt p) T -> p dt T", p=128)

        bulk_dmas = []
        for _rep in range(repeat):
            for g in range(N_GROUP):
                t0 = g * GTOK
                xthi = xt_pool.tile([128, D_TILES, GTOK], f16, name="xthi")
                nc.sync.dma_start(xthi[:], xthi_v[:, :, t0 : t0 + GTOK])
                for half in range(2):
                    hs = slice(half * 512, half * 512 + 512)
                    h8 = ps_h.tile([K, 512], f32, name="h8")
                    for dt in range(D_TILES):
                        nc.tensor.matmul(
                            h8[:],
                            lhsT=whi_sb[:, dt * K : (dt + 1) * K],
                            rhs=xthi[:, dt, hs],
                            start=(dt == 0),
                            stop=(dt == D_TILES - 1),
                        )
                    # val+ (thresholds - eps) rows 0:8, val- (+eps) rows 8:16
                    u2 = val_pool.tile([40, 512], f32, name="u2")
                    v1p = val_pool.tile([K, 512], f32, name="v1p")
                    nc.vector.tensor_scalar(
                        out=v1p[:], in0=h8[:], scalar1=thr4_sb[:, 0:1],
                        scalar2=None, op0=mybir.AluOpType.is_ge,
                    )
                    nc.vector.scalar_tensor_tensor(
                        out=u2[0:K, :], in0=h8[:], scalar=thr4_sb[:, 1:2],
                        in1=v1p[:], op0=mybir.AluOpType.is_ge,
                        op1=mybir.AluOpType.add,
                    )
                    v1m = val_pool.tile([K, 512], f32, name="v1p")
                    nc.vector.tensor_scalar(
                        out=v1m[:], in0=h8[:], scalar1=thr4_sb[:, 2:3],
                        scalar2=None, op0=mybir.AluOpType.is_ge,
                    )
                    nc.vector.scalar_tensor_tensor(
                        out=u2[32 : 32 + K, :], in0=h8[:], scalar=thr4_sb[:, 3:4],
                        in1=v1m[:], op0=mybir.AluOpType.is_ge,
                        op1=mybir.AluOpType.add,
                    )
                    # row 0 = mu+, row 32 = mu+ - mu- (flag diff, >= 0)
                    mu2 = ps_m.tile([33, 512], f32, name="mu2")
                    nc.tensor.matmul(
                        mu2[:], lhsT=pw2_sb[:], rhs=u2[:], start=True, stop=True
                    )
                    base = t0 + half * 512
                    muh = val_pool.tile([1, 512], i32, name="muh")
                    nc.vector.tensor_copy(muh[:], mu2[0:1, :])
                    cb = base // 4
                    nc.vector.tensor_reduce(
                        out=cf_all[:, cb : cb + 128],
                        in_=mu2[32:33, :].rearrange(
                            "one (c four) -> one c four", four=4
                        ),
                        axis=mybir.AxisListType.X,
                        op=mybir.AluOpType.max,
                    )
                    bd = nc.scalar.dma_start(
                        out_ap[:, base : base + 512], muh[:]
                    )
                    tc.chain_iter_dep("outwaw", bd.ins)

        # ---- compaction (chunk domain, probe-verified op sequence) ----
        cfb = cmp_pool.tile([1, NCHUNK], f32)
        nc.vector.tensor_scalar(
            out=cfb[:], in0=cf_all[:], scalar1=0.5, scalar2=None,
            op0=mybir.AluOpType.is_gt,
        )
        nc.vector.memset(cfb[:, 0:1], 1.0)
        S = cmp_pool.tile([1, NCHUNK], f32)
        nc.vector.tensor_tensor_scan(
            S[:], cfb[:], cfb[:], 0.0,
            mybir.AluOpType.add, mybir.AluOpType.bypass,
        )
        # slot+1 of flagged chunk c is S[c]*cfb[c]; jcol is host-shifted by +1
        slotsel = cmp_pool.tile([1, NCHUNK], f32)
        nc.vector.tensor_mul(slotsel[:], S[:], cfb[:])
        cidp = cmp_pool.tile([128, 2, 4], f32)
        em = valb_pool.tile([128, 512], f32, name="em")
        junk = valb_pool.tile([128, 512], f32, name="junk")
        for b in range(2):
            for q in range(NCHUNK // 512):
                sl = slice(q * 512, (q + 1) * 512)
                bcs = ps_c.tile([128, 512], f32, name="bcs")
                nc.tensor.matmul(
                    bcs[:], lhsT=ones128[:], rhs=slotsel[:, sl],
                    start=True, stop=True,
                )
                bci = ps_c.tile([128, 512], f32, name="bci")
                nc.tensor.matmul(
                    bci[:], lhsT=ones128[:], rhs=iop1[:, sl],
                    start=True, stop=True,
                )
                nc.vector.tensor_scalar(
                    out=em[:], in0=bcs[:], scalar1=jcol[:, b : b + 1],
                    scalar2=None, op0=mybir.AluOpType.is_equal,
                )
                nc.vector.scalar_tensor_tensor(
                    out=junk[:], in0=bci[:], scalar=1.0, in1=em[:],
                    op0=mybir.AluOpType.mult, op1=mybir.AluOpType.mult,
                    accum_out=cidp[:, b, q : q + 1],
                )
        cidsum = cmp_pool.tile([128, 2], f32)
        nc.vector.tensor_reduce(
            out=cidsum[:], in_=cidp[:], axis=mybir.AxisListType.X,
            op=mybir.AluOpType.add,
        )
        cidm1 = cmp_pool.tile([128, 2], f32)
        nc.vector.tensor_scalar(
            out=cidm1[:], in0=cidsum[:], scalar1=-1.0, scalar2=None,
            op0=mybir.AluOpType.add,
        )
        nc.sync.dma_start(dbg_ap[:], cidm1[:])
        # pads (-1) -> BIGIDX so bounds_check skips them in gather and scatter
        tneg = cmp_pool.tile([128, 2], f32)
        nc.vector.tensor_scalar(
            out=tneg[:], in0=cidm1[:], scalar1=0.0, scalar2=None,
            op0=mybir.AluOpType.is_lt,
        )
        cidB_f = cmp_pool.tile([128, 2], f32)
        nc.vector.scalar_tensor_tensor(
            out=cidB_f[:], in0=tneg[:], scalar=float(BIGIDX + 1), in1=cidm1[:],
            op0=mybir.AluOpType.mult, op1=mybir.AluOpType.add,
        )
        cidBIG = cmp_pool.tile([128, 2], i32)
        nc.vector.tensor_copy(cidBIG[:], cidB_f[:])

        # ---- pass B: gather flagged chunks, recompute exactly, scatter ----
        for b in range(2):
            xg = xg_pool.tile([128, 4 * 2 * D], f16, name="xg")
            gi = nc.gpsimd.indirect_dma_start(
                out=xg[:],
                out_offset=None,
                in_=xpair_ap[:],
                in_offset=_bass.IndirectOffsetOnAxis(
                    ap=cidBIG[:, b : b + 1], axis=0
                ),
                bounds_check=NCHUNK - 1,
                oob_is_err=False,
            )
            # transpose chunk-major rows to d-on-partitions:
            # element m = tc*2560 + dt2*128 + p  (dt2<10: hi, >=10: lo)
            xgT = xgt_pool.tile([128, 2 * D_TILES, 4, 128], f16, name="xgT")
            for e in range(80):
                tc_i, dt2 = divmod(e, 20)
                tp = ps_t.tile([128, 128], f16, name="tp16")
                nc.tensor.transpose(
                    tp[:], xg[:, e * 128 : (e + 1) * 128], ident16[:]
                )
                dst = xgT[:, dt2, tc_i, :]
                if e % 2 == 0:
                    nc.vector.tensor_copy(dst, tp[:])
                else:
                    nc.scalar.copy(dst, tp[:])
            h40 = ps_c.tile([WP, 512], f32, name="h40B")
            for dt2 in range(2 * D_TILES):
                nc.tensor.matmul(
                    h40[:],
                    lhsT=wpair_sb[:, (dt2 % D_TILES) * WP : (dt2 % D_TILES + 1) * WP],
                    rhs=xgT[:, dt2, :, :],
                    start=(dt2 == 0),
                    stop=(dt2 == 2 * D_TILES - 1),
                )
            hlo_sb = valb_pool.tile([K, 512], f32, name="hlo_sb")
            nc.vector.tensor_copy(hlo_sb[:], h40[32 : 32 + K, :])
            hsum = valb_pool.tile([K, 512], f32, name="hsum")
            nc.vector.tensor_add(hsum[:], h40[0:K, :], hlo_sb[:])
            vb1 = valb_pool.tile([K, 512], f32, name="vb1")
            nc.vector.tensor_scalar(
                out=vb1[:], in0=hsum[:], scalar1=thr2_sb[:, 0:1],
                scalar2=None, op0=mybir.AluOpType.is_ge,
            )
            vb = valb_pool.tile([K, 512], f32, name="vb")
            nc.vector.scalar_tensor_tensor(
                out=vb[:], in0=hsum[:], scalar=thr2_sb[:, 1:2], in1=vb1[:],
                op0=mybir.AluOpType.is_ge, op1=mybir.AluOpType.add,
            )
            mu2b = ps_m.tile([33, 512], f32, name="mu2")
            nc.tensor.matmul(
                mu2b[0:1, :], lhsT=pw2_sb[0:K, 0:1], rhs=vb[:],
                start=True, stop=True,
            )
            muI = valb_pool.tile([1, 512], i32, name="muI")
            nc.vector.tensor_copy(muI[:], mu2b[0:1, :])
            wr = nc.sync.dma_start(muscr_ap[b : b + 1, :], muI[:])
            tc.chain_iter_dep(f"muscr{b}", wr.ins)
            valsI = valb_pool.tile([128, 4], i32, name="valsI")
            rd = nc.sync.dma_start(
                valsI[:], muscr_ap[b, :].rearrange("(tc j) -> j tc", j=128)
            )
            tc.chain_iter_dep(f"muscr{b}", rd.ins)
            sc = nc.gpsimd.indirect_dma_start(
                out=out_ap.rearrange("one (c four) -> (one c) four", four=4),
                out_offset=_bass.IndirectOffsetOnAxis(
                    ap=cidBIG[:, b : b + 1], axis=0
                ),
                in_=valsI[:],
                in_offset=None,
                bounds_check=NCHUNK - 1,
                oob_is_err=False,
            )
            tc.chain_iter_dep("outwaw", sc.ins)

    nc.compile()
    return nc


MODE = "fp16x2"


def _build_program(repeat=1):
    if MODE == "twopass":
        return _build_twopass(repeat)
    return _build_fp16x2(repeat)


def _get_program(repeat=1):
    key = ("nc", repeat)
    if key not in _cached:
        _cached[key] = _build_program(repeat)
    return _cached[key]


def _split_f16(a32):
    hi = a32.astype(np.float16)
    lo = (a32 - hi.astype(np.float32)).astype(np.float16)
    return hi, lo


def _min_f32_ge(B, T):
    """Minimal f32 v with fl32(v + B) >= T (B, T f32). Monotone bisection."""
    B = np.float32(B)
    T = np.float32(T)

    def f(v):
        return np.float32(v) + B >= T

    lo, hi = np.float64(-1e9), np.float64(1e9)
    assert not f(np.float32(lo)) and f(np.float32(hi))
    for _ in range(200):
        mid = (lo + hi) / 2
        if f(np.float32(mid)):
            hi = mid
        else:
            lo = mid
    v = np.float32(hi)
    while f(np.float32(np.nextafter(v, np.float32(-np.inf), dtype=np.float32))):
        v = np.nextafter(v, np.float32(-np.inf), dtype=np.float32)
    assert f(v)
    return v


def make_in_maps(x, W, b):
    if MODE == "twopass":
        return _make_in_maps_twopass(x, W, b)
    xf = np.ascontiguousarray(x.reshape(-1, D), dtype=np.float32)
    powers = (3.0 ** np.arange(K, dtype=np.float32)).reshape(K, 1).astype(np.float32)
    xs = xf * np.float32(SPLIT_SCALE)
    xhi, xlo = _split_f16(xs)
    ws = np.ascontiguousarray(W.T, dtype=np.float32) * np.float32(SPLIT_SCALE)
    wthi, wtlo = _split_f16(ws)
    # prebuilt stacked stationary image: [128, 10 d-tiles * 40 cols],
    # cols 0:8 = Whi_dt, cols 32:40 = Wlo_dt
    wpair = np.zeros((128, D_TILES * 40), np.float16)
    for dt in range(D_TILES):
        wpair[:, dt * 40 : dt * 40 + K] = wthi[dt * 128 : (dt + 1) * 128]
        wpair[:, dt * 40 + 32 : dt * 40 + 40] = wtlo[dt * 128 : (dt + 1) * 128]
    # fold bias into exact per-k thresholds on the scaled h:
    #   [fl32(h + B_k) >= T_HI]  <=>  [h >= tpos_k]
    #   [fl32(h + B_k) > -T_HI]  <=>  [h >= tneg_k]
    bs = (b.reshape(K).astype(np.float32)) * np.float32(SPLIT_SCALE * SPLIT_SCALE)
    T_HI = np.float32(T_POS) * np.float32(SPLIT_SCALE * SPLIT_SCALE)
    succ_negT = np.nextafter(-T_HI, np.float32(np.inf), dtype=np.float32)
    tpos = np.array([_min_f32_ge(bs[k], T_HI) for k in range(K)], np.float32)
    tneg = np.array([_min_f32_ge(bs[k], succ_negT) for k in range(K)], np.float32)
    return [
        {
            "xtall": np.ascontiguousarray(
                np.concatenate(
                    [
                        xhi[c * TOK_PER_CORE : (c + 1) * TOK_PER_CORE].T,
                        xlo[c * TOK_PER_CORE : (c + 1) * TOK_PER_CORE].T,
                    ],
                    axis=0,
                )
            ),
            "wpair": wpair,
            "tpos": tpos.reshape(K, 1),
            "tneg": tneg.reshape(K, 1),
            "powers": powers,
        }
        for c in range(N_CORES)
    ]


def _make_in_maps_twopass(x, W, b):
    xf = np.ascontiguousarray(x.reshape(-1, D), dtype=np.float32)
    xs = xf * np.float32(SPLIT_SCALE)
    xhi, xlo = _split_f16(xs)
    ws = np.ascontiguousarray(W.T, dtype=np.float32) * np.float32(SPLIT_SCALE)
    wthi, wtlo = _split_f16(ws)
    bs = (b.reshape(K).astype(np.float32)) * np.float32(SPLIT_SCALE * SPLIT_SCALE)
    T_HI = np.float32(T_POS) * np.float32(SPLIT_SCALE * SPLIT_SCALE)
    succ_negT = np.nextafter(-T_HI, np.float32(np.inf), dtype=np.float32)
    tpos = np.array([_min_f32_ge(bs[k], T_HI) for k in range(K)], np.float32)
    tneg = np.array([_min_f32_ge(bs[k], succ_negT) for k in range(K)], np.float32)
    e = np.float32(EPS_S)
    thr4 = np.stack([tpos - e, tneg - e, tpos + e, tneg + e], axis=1)
    thr2 = np.stack([tpos, tneg], axis=1)
    pw = 3.0 ** np.arange(K, dtype=np.float32)
    pw2 = np.zeros((40, 33), np.float32)
    pw2[0:K, 0] = pw       # col 0: mu+ (val+ rows)
    pw2[0:K, 32] = pw      # col 32: mu+ - mu-
    pw2[32 : 32 + K, 32] = -pw
    iop1 = (np.arange(NCHUNK, dtype=np.float32) + 1.0).reshape(1, NCHUNK)
    # em compares S*f (slot+1) against j+1: shift jcol by +1
    jcol = (
        np.arange(128, dtype=np.float32)[:, None] + np.array([[1.0, 129.0]])
    ).astype(np.float32)
    maps = []
    for c in range(N_CORES):
        sl = slice(c * TOK_PER_CORE, (c + 1) * TOK_PER_CORE)
        xhi_c = xhi[sl]
        xlo_c = xlo[sl]
        xpair = np.ascontiguousarray(
            np.concatenate([xhi_c, xlo_c], axis=1).reshape(NCHUNK, 4 * 2 * D)
        )
        maps.append(
            {
                "xthi": np.ascontiguousarray(xhi_c.T),
                "xpair": xpair,
                "wthi": wthi,
                "wtlo": wtlo,
                "thr4": np.ascontiguousarray(thr4),
                "thr2": np.ascontiguousarray(thr2),
                "pw2": pw2,
                "iop1": iop1,
                "jcol": jcol,
            }
        )
    return maps


def kernel(x: np.ndarray, W: np.ndarray, b: np.ndarray) -> np.ndarray:
    from concourse.bass_utils import run_bass_kernel_spmd

    nc = _get_program()

    B, T, Dx = x.shape
    assert (B * T, Dx) == (N_CORES * TOK_PER_CORE, D)
    in_maps = make_in_maps(x, W, b)
    res = run_bass_kernel_spmd(nc, in_maps, list(range(N_CORES)))
    mu = np.concatenate(
        [res.results[c]["out"].reshape(-1) for c in range(N_CORES)]
    )
    return mu.reshape(B, T).astype(np.int32)
